# revision 3
# baseline (speedup 1.0000x reference)
"""Trainium2 Bass kernel for a Tsit5 NeuralODE (MLP vector field).

Contract: kernel(**inputs) takes the FULL inputs of reference.setup_inputs()
and returns the FULL [101, 4096, 64] trajectory. Internally: data-parallel
shard of the batch across 8 NeuronCores; each core integrates its 512 rows
through 100 Tsit5 steps (6 MLP evals per step) with:
  - matmuls in float32r (TF32-like, full PE rate at N>=256, ~1e-3 rel err)
  - tanh (+ per-channel bias) on ScalarE reading PSUM directly
  - Runge-Kutta stage combinations as fused (k*A_ij)+acc scalar_tensor_tensor
    ops on VectorE; the y-update chain runs on GpSimdE
  - h_t and h_t*b3 folded into the PSUM->SBUF k-evacuation (one tensor_scalar)
  - activations kept transposed [dim, batch]; batch split in 2 chains of 256
    so PE can work on one chain while ScalarE tanh's the other.
Host side: transpose/shard/gather + tiny tables (h_t broadcast) in numpy.
"""

import numpy as np

import concourse.bass as bass
import concourse.tile as tile
from concourse import bacc, mybir
from concourse.bass_utils import run_bass_kernel_spmd

# Tsit5 tableau (must match the reference)
A21 = 0.161
A31, A32 = -0.008480655492356989, 0.335480655492357
A41, A42, A43 = 2.8971530571054935, -6.359448489975075, 4.3622954328695815
A51, A52, A53, A54 = 5.325864828439257, -11.748883564062828, 7.4955393428898365, -0.09249506636175525
A61, A62, A63, A64, A65 = 5.86145544294642, -12.92096931784711, 8.159367898576159, -0.071584973281401, -0.028269050394068383
B1, B2, B3, B4, B5, B6 = 0.09646076681806523, 0.01, 0.4798896504144996, 1.379008574103742, -3.290069515436081, 2.324710524099774

# A[i][j] = coefficient of k_j in stage i's input (stages 2..6)
ACOEF = {
    2: {1: A21},
    3: {1: A31, 2: A32},
    4: {1: A41, 2: A42, 3: A43},
    5: {1: A51, 2: A52, 3: A53, 4: A54},
    6: {1: A61, 2: A62, 3: A63, 4: A64, 5: A65},
}
BCOEF = {1: B1, 2: B2, 3: B3, 4: B4, 5: B5, 6: B6}

NCORES = 8
DIM, WIDTH = 64, 256
BATCH, NT = 4096, 101
NSTEP = NT - 1
SHARD = BATCH // NCORES      # 512 rows per core
CH = 2                       # batch chains per core
CN = SHARD // CH             # 256 = chain width (also min N for f32r full rate)

F32 = mybir.dt.float32
F32R = mybir.dt.float32r
MULT = mybir.AluOpType.mult
ADD = mybir.AluOpType.add
TANH = mybir.ActivationFunctionType.Tanh

_cache = {}


def _build(nsteps=NSTEP):
    nc = bacc.Bacc("TRN2", target_bir_lowering=False, debug=False, num_devices=NCORES)

    y0t_d = nc.dram_tensor("y0t", [DIM, SHARD], F32, kind="ExternalInput").ap()
    hb_d = nc.dram_tensor("hb", [DIM, nsteps], F32, kind="ExternalInput").ap()
    hb3_d = nc.dram_tensor("hb3", [DIM, nsteps], F32, kind="ExternalInput").ap()
    w0_d = nc.dram_tensor("W0", [DIM, WIDTH], F32, kind="ExternalInput").ap()
    w1_d = nc.dram_tensor("W1", [WIDTH, WIDTH], F32, kind="ExternalInput").ap()
    w2_d = nc.dram_tensor("W2", [WIDTH, WIDTH], F32, kind="ExternalInput").ap()
    w3_d = nc.dram_tensor("W3", [WIDTH, DIM], F32, kind="ExternalInput").ap()
    b0_d = nc.dram_tensor("b0", [WIDTH], F32, kind="ExternalInput").ap()
    b1_d = nc.dram_tensor("b1", [WIDTH], F32, kind="ExternalInput").ap()
    b2_d = nc.dram_tensor("b2", [WIDTH], F32, kind="ExternalInput").ap()
    out_d = nc.dram_tensor("ysT", [nsteps, DIM, SHARD], F32, kind="ExternalOutput").ap()

    with tile.TileContext(nc) as tc:
        with tc.tile_pool(name="const", bufs=1) as const, \
             tc.tile_pool(name="state", bufs=2) as state, \
             tc.tile_pool(name="work", bufs=3) as work, \
             tc.tile_pool(name="psum", bufs=1, space="PSUM") as psum:

            # ---- load + round weights to f32r ----
            w0s = const.tile([DIM, 2, 128], F32, tag="w0s")
            nc.sync.dma_start(w0s[:], w0_d.rearrange("k (m j) -> k m j", j=128))
            w0 = const.tile([DIM, 2, 128], F32R, tag="w0")
            nc.vector.tensor_copy(w0[:], w0s[:])

            w1 = const.tile([128, 2, 2, 128], F32R, tag="w1")
            w2 = const.tile([128, 2, 2, 128], F32R, tag="w2")
            for wd, wt, nm in ((w1_d, w1, "w1"), (w2_d, w2, "w2")):
                ws = const.tile([128, 2, 2, 128], F32, tag=nm + "s", name=nm + "s")
                for t in range(2):
                    nc.sync.dma_start(
                        ws[:, t],
                        wd[t * 128:(t + 1) * 128, :].rearrange("k (m j) -> k m j", j=128),
                    )
                nc.vector.tensor_copy(wt[:], ws[:])

            w3s = const.tile([128, 2, DIM], F32, tag="w3s")
            nc.sync.dma_start(w3s[:], w3_d.rearrange("(t k) d -> k t d", k=128))
            w3 = const.tile([128, 2, DIM], F32R, tag="w3")
            nc.vector.tensor_copy(w3[:], w3s[:])

            # ---- biases as [128, 2] (column m = Mtile m) ----
            bt = {}
            for bd, nm in ((b0_d, "b0"), (b1_d, "b1"), (b2_d, "b2")):
                tile_b = const.tile([128, 2], F32, tag=nm + "t", name=nm + "t")
                nc.sync.dma_start(tile_b[:], bd.rearrange("(m p) -> p m", p=128))
                bt[nm] = tile_b

            # ---- per-step scalar tables ----
            hb = const.tile([DIM, nsteps], F32, tag="hb")
            nc.sync.dma_start(hb[:], hb_d)
            hb3 = const.tile([DIM, nsteps], F32, tag="hb3")
            nc.sync.dma_start(hb3[:], hb3_d)

            # ---- initial state ----
            y = state.tile([DIM, SHARD], F32, tag="y", name="y")
            nc.sync.dma_start(y[:], y0t_d)
            yr = state.tile([DIM, SHARD], F32R, tag="yr", name="yr")
            nc.vector.tensor_copy(yr[:], y[:])

            for t in range(nsteps):
                h_ap = hb[:, t:t + 1]
                hb3_ap = hb3[:, t:t + 1]

                khat = {}
                acc = {i: None for i in range(3, 7)}
                accy = None
                z = {}
                y_next = None

                for s in range(1, 7):
                    rhs = yr if s == 1 else z[s]

                    # ---- MLP eval on transposed activations, per chain ----
                    ps0 = [psum.tile([128, CH, CN], F32, tag=f"ps0_{m}", name=f"ps0_{m}") for m in range(2)]
                    for c in range(CH):
                        cs = slice(c * CN, (c + 1) * CN)
                        for m in range(2):
                            nc.tensor.matmul(ps0[m][:, c], w0[:, m], rhs[:, cs],
                                             start=True, stop=True)
                    h0 = [work.tile([128, CH, CN], F32R, tag=f"h0_{m}", name=f"h0_{m}") for m in range(2)]
                    for m in range(2):
                        nc.scalar.activation(h0[m][:], ps0[m][:], TANH,
                                             bias=bt["b0"][:, m:m + 1])

                    ps1 = [psum.tile([128, CH, CN], F32, tag=f"ps1_{m}", name=f"ps1_{m}") for m in range(2)]
                    for c in range(CH):
                        for m in range(2):
                            nc.tensor.matmul(ps1[m][:, c], w1[:, 0, m], h0[0][:, c],
                                             start=True, stop=False)
                            nc.tensor.matmul(ps1[m][:, c], w1[:, 1, m], h0[1][:, c],
                                             start=False, stop=True)
                    h1 = [work.tile([128, CH, CN], F32R, tag=f"h1_{m}", name=f"h1_{m}") for m in range(2)]
                    for m in range(2):
                        nc.scalar.activation(h1[m][:], ps1[m][:], TANH,
                                             bias=bt["b1"][:, m:m + 1])

                    ps2 = [psum.tile([128, CH, CN], F32, tag=f"ps2_{m}", name=f"ps2_{m}") for m in range(2)]
                    for c in range(CH):
                        for m in range(2):
                            nc.tensor.matmul(ps2[m][:, c], w2[:, 0, m], h1[0][:, c],
                                             start=True, stop=False)
                            nc.tensor.matmul(ps2[m][:, c], w2[:, 1, m], h1[1][:, c],
                                             start=False, stop=True)
                    h2 = [work.tile([128, CH, CN], F32R, tag=f"h2_{m}", name=f"h2_{m}") for m in range(2)]
                    for m in range(2):
                        nc.scalar.activation(h2[m][:], ps2[m][:], TANH,
                                             bias=bt["b2"][:, m:m + 1])

                    ps3 = psum.tile([DIM, CH, CN], F32, tag="ps3", name="ps3")
                    for c in range(CH):
                        nc.tensor.matmul(ps3[:, c], w3[:, 0], h2[0][:, c],
                                         start=True, stop=False)
                        nc.tensor.matmul(ps3[:, c], w3[:, 1], h2[1][:, c],
                                         start=False, stop=True)

                    # ---- Khat_s = h*(L3out + b3) : PSUM->SBUF, per chain ----
                    kh = work.tile([DIM, SHARD], F32, tag=f"khat{s}", name=f"khat{s}")
                    for c in range(CH):
                        cs = slice(c * CN, (c + 1) * CN)
                        nc.vector.tensor_scalar(kh[:, cs], ps3[:, c], h_ap, hb3_ap,
                                                MULT, ADD)
                    khat[s] = kh

                    # ---- fold Khat_s into future stage inputs (VectorE) ----
                    for i in range(s + 1, 7):
                        a_is = ACOEF[i][s]
                        last = (s == i - 1)
                        if last:
                            dst = work.tile([DIM, SHARD], F32R, tag=f"z{i}", name=f"z{i}")
                        else:
                            dst = acc[i] if acc[i] is not None else \
                                work.tile([DIM, SHARD], F32, tag=f"acc{i}", name=f"acc{i}")
                        src1 = y if s == 1 else acc[i]
                        for c in range(CH):
                            cs = slice(c * CN, (c + 1) * CN)
                            nc.vector.scalar_tensor_tensor(
                                dst[:, cs], kh[:, cs], a_is, src1[:, cs], MULT, ADD)
                        if last:
                            z[i] = dst
                        else:
                            acc[i] = dst

                    # ---- fold Khat_s into the y update (GpSimdE) ----
                    if s < 6:
                        dsty = accy if accy is not None else \
                            work.tile([DIM, SHARD], F32, tag="accy", name="accy")
                        srcy = y if s == 1 else accy
                    else:
                        dsty = state.tile([DIM, SHARD], F32, tag="y", name="y")
                        srcy = accy
                    for c in range(CH):
                        cs = slice(c * CN, (c + 1) * CN)
                        nc.vector.scalar_tensor_tensor(
                            dsty[:, cs], kh[:, cs], BCOEF[s], srcy[:, cs], MULT, ADD)
                    if s < 6:
                        accy = dsty
                    else:
                        y_next = dsty

                # ---- commit step ----
                nc.sync.dma_start(out_d[t], y_next[:])
                y = y_next
                yr = state.tile([DIM, SHARD], F32R, tag="yr", name="yr")
                nc.vector.tensor_copy(yr[:], y[:])

    nc.compile()
    return nc


def _get_nc(nsteps=NSTEP):
    if nsteps not in _cache:
        _cache[nsteps] = _build(nsteps)
    return _cache[nsteps]


def _prepare_in_maps(ts, y0, W0, b0, W1, b1, W2, b2, W3, b3, nsteps=NSTEP):
    ts = np.asarray(ts, np.float32)
    hs = (ts[1:nsteps + 1] - ts[:nsteps]).astype(np.float32)          # [nsteps]
    hb = np.broadcast_to(hs[None, :], (DIM, nsteps)).copy()           # h_t
    hb3 = (hs[None, :] * np.asarray(b3, np.float32)[:, None]).copy()  # h_t*b3[d]
    common = {
        "hb": hb, "hb3": hb3,
        "W0": np.ascontiguousarray(W0, np.float32),
        "W1": np.ascontiguousarray(W1, np.float32),
        "W2": np.ascontiguousarray(W2, np.float32),
        "W3": np.ascontiguousarray(W3, np.float32),
        "b0": np.ascontiguousarray(b0, np.float32),
        "b1": np.ascontiguousarray(b1, np.float32),
        "b2": np.ascontiguousarray(b2, np.float32),
    }
    in_maps = []
    for i in range(NCORES):
        shard = np.asarray(y0[i * SHARD:(i + 1) * SHARD], np.float32)
        in_maps.append({"y0t": np.ascontiguousarray(shard.T), **common})
    return in_maps


def _run(inputs, nsteps=NSTEP, trace=False):
    nc = _get_nc(nsteps)
    in_maps = _prepare_in_maps(**inputs, nsteps=nsteps)
    res = run_bass_kernel_spmd(nc, in_maps, core_ids=list(range(NCORES)), trace=trace)
    y0 = np.asarray(inputs["y0"], np.float32)
    out = np.empty((nsteps + 1, BATCH, DIM), np.float32)
    out[0] = y0
    for i in range(NCORES):
        out[1:, i * SHARD:(i + 1) * SHARD, :] = res.results[i]["ysT"].transpose(0, 2, 1)
    return out, res


def kernel(**inputs) -> np.ndarray:
    out, _ = _run(inputs)
    return out
